# revision 25
# baseline (speedup 1.0000x reference)
"""Talking-heads causal attention kernel for 8 Trainium2 NeuronCores.

Problem: B=4, H=16, N=1024, D=64 (fp32)
  dots = einsum('bhid,bhjd', q, k) * d**-0.5
  dots = einsum('gh,bhij', w_pre, dots) + attn_bias   (talking heads pre)
  causal mask, fp32 softmax
  attn = einsum('gh,bhij', w_post, attn)              (talking heads post)
  out  = einsum('bhij,bhjd', attn, v)
Sharding: core c = (b, s) with b = c//2, s = c%2. Each core owns query rows
R_s = {128k + 64s + [0,64) : k=0..7} of its batch b (interleaved 64-row
blocks -> identical causal work AND identical program on every core).
The h-mixes are local (all 16 heads on-core); no collectives.

Device pipeline per core (pairs m=0..3 of row-groups, 128 rows each):
  QK^T (f16)    ->  dots in natural [i,(h,j)] layout (PSUM -> dnat SBUF,
                    evac spread over DVE/Act/Pool engines)
  DMA shuffle   ->  [(i8,h), j] interleaved layout (8->128 partition DMA, SP)
  bias via identity-matmul into PSUM + pre-mix Kronecker matmul (I8 (x) w_pre)
  ScalarE exp(x-4) with fused row-sum accum
  post-mix+transpose+normalize as ONE matmul: lhsT=E chunk, rhs=R where
     R = (I8 (x) w_post^T) * (1/S) rowwise  ->  out = attn_mixed^T [j,(i8,g)]
     PSUM evacuated in batched 512-col copies
  AV matmul (fp16) with strided lhsT gather, accumulate over j chunks,
  in two 8-head halves sharing one PSUM bank; av -> out_t f16, DMA on SP.
"""

import numpy as np
import ml_dtypes

B, H, N, D = 4, 16, 1024, 64
N_CORES = 8
NBLK = 16          # 8-row blocks per 128-row pair-group
NPAIR = 4          # pair-groups per core (each 128 rows = 16 blks)

MASK_VAL = np.float32(-60000.0)
EXP_SHIFT = -4.0

# engine-assignment patterns (cycled): v=vector(DVE) s=scalar(Act) g=gpsimd(Pool)
QK_EVAC_PAT = "vvvs"
TP_EVAC_PAT = "vvvs"
OUT_ENG = "sv"
MSEQ = (0, 1, 2, 3)


def _core_rows(s):
    """Global row indices (length 512) owned by core (b, s), pair-major."""
    rows = []
    for m in range(NPAIR):
        for k in (2 * m, 2 * m + 1):
            base = 128 * k + 64 * s
            rows.extend(range(base, base + 64))
    return np.array(rows)  # [512]; pair m -> rows[m*128:(m+1)*128]


def _pair_ext(m, blk):
    """#128-wide j-chunks needed by 8-row block blk of pair m (causal)."""
    k = 2 * m + (blk // 8)          # which 64-row group
    return k + 1


def _quarter_F(m, q):
    """j-extent (cols) of bias quarter q (blocks 4q..4q+3) of pair m."""
    return 128 * (2 * m + q // 2 + 1)


def _build_module(qk_evac=QK_EVAC_PAT, tp_evac=TP_EVAC_PAT, out_eng=OUT_ENG,
                  mseq=MSEQ, qk_bufs=2, pm_bufs=2, exp_split=False,
                  EARLY_SHUF=False, FRONT=13, FIRST_PM=False):
    import concourse.bass as bass
    import concourse.mybir as mybir
    import concourse.tile as tile
    from concourse import bacc

    f32, f16 = mybir.dt.float32, mybir.dt.float16

    nc = bacc.Bacc("TRN2", target_bir_lowering=False, debug=False,
                   num_devices=N_CORES)

    # q/k transposed, two heads packed per partition-column: head h lives at
    # partitions (h%2)*64 + d, free index h//2.  qT pair-major for split loads.
    qT_ap = nc.dram_tensor("qT", [128, NPAIR, H // 2, 128], f16, kind="ExternalInput").ap()
    kT_ap = nc.dram_tensor("kT", [128, H // 2, N], f16, kind="ExternalInput").ap()
    v_ap = nc.dram_tensor("v", [128, 8, H, 64], f16, kind="ExternalInput").ap()
    # bias per (pair, quarter): blocks 4q..4q+3 shuffled to [(i8,h), (blk4, j)]
    bias_aps = {}
    for m in range(NPAIR):
        for q in range(4):
            Fq = _quarter_F(m, q)
            bias_aps[(m, q)] = nc.dram_tensor(
                f"bias{m}_{q}", [128, 4, Fq], f16, kind="ExternalInput").ap()
    wpre_ap = nc.dram_tensor("wpre", [128, 128], f16, kind="ExternalInput").ap()
    wpost_ap = nc.dram_tensor("wpost", [128, 128], f32, kind="ExternalInput").ap()
    ident_ap = nc.dram_tensor("ident", [128, 128], f16, kind="ExternalInput").ap()
    out_ap = nc.dram_tensor("out", [NPAIR, 128, H, 64], f16, kind="ExternalOutput").ap()

    with tile.TileContext(nc) as tc:
        with (
            tc.tile_pool(name="const", bufs=1) as cpool,
            tc.tile_pool(name="dnat", bufs=1) as dnat_pool,
            tc.tile_pool(name="dshuf", bufs=4) as dshuf_pool,
            tc.tile_pool(name="ebuf", bufs=4) as e_pool,
            tc.tile_pool(name="et", bufs=1) as et_pool,
            tc.tile_pool(name="biasb", bufs=4) as bias_pool,
            tc.tile_pool(name="small", bufs=4) as s_pool,
            tc.tile_pool(name="outb", bufs=2) as out_pool,
            tc.tile_pool(name="qkps", bufs=qk_bufs, space="PSUM") as qk_psum,
            tc.tile_pool(name="pmps", bufs=pm_bufs, space="PSUM") as pm_psum,
            tc.tile_pool(name="tpps", bufs=2, space="PSUM") as tp_psum,
            # pm tiles are [128,1024] (2 banks) unless exp_split
        ):
            Exp = mybir.ActivationFunctionType.Exp
            ENG = {}

            def copy_on(key, dst, src):
                eng = ENG[key]
                if eng is nc.scalar:
                    eng.copy(dst, src)
                else:
                    eng.tensor_copy(dst, src)

            ENG.update(v=nc.vector, g=nc.gpsimd)
            ENG['s'] = nc.scalar

            # --- constants / inputs, ordered for fast pipeline start: tiny
            # consts on SP (fast HWDGE gen), kT chunk then qT(first pair) on
            # Pool so the first QK + premix unblock ASAP; v late and split.
            m0 = mseq[0]
            wpre = cpool.tile([128, 128], f16, tag="wpre")
            nc.sync.dma_start(wpre[:], wpre_ap[:])
            ident = cpool.tile([128, 128], f16, tag="ident")
            nc.sync.dma_start(ident[:], ident_ap[:])
            kT = cpool.tile([128, H // 2, N], f16, tag="kT")
            F0 = 128 * (2 * m0 + 2)
            nc.gpsimd.dma_start(kT[:, :, 0:F0], kT_ap[:, :, 0:F0])
            qT = cpool.tile([128, NPAIR, H // 2, 128], f16, tag="qT")
            nc.gpsimd.dma_start(qT[:, m0], qT_ap[:, m0])
            wpost = cpool.tile([128, 128], f32, tag="wpost")
            nc.sync.dma_start(wpost[:], wpost_ap[:])
            shift = cpool.tile([128, 1], f32, tag="shift")
            nc.vector.memset(shift[:], EXP_SHIFT)

            bias_tiles = {}

            def issue_bias(m, q):
                Fq = _quarter_F(m, q)
                bt = bias_pool.tile([128, 4, 1024], f16, tag="bias",
                                    name=f"bias{m}_{q}")
                nc.sync.dma_start(bt[:, :, :Fq], bias_aps[(m, q)][:])
                bias_tiles[(m, q)] = bt

            issue_bias(m0, 0)
            for mm in mseq[1:]:
                nc.gpsimd.dma_start(qT[:, mm], qT_ap[:, mm])
            issue_bias(m0, 1)
            kT1 = min(512, N)
            if F0 < kT1:
                nc.gpsimd.dma_start(kT[:, :, F0:kT1], kT_ap[:, :, F0:kT1])
            v_sb = cpool.tile([128, 8, H, 64], f16, tag="v")
            nc.sync.dma_start(v_sb[:, 0:2], v_ap[:, 0:2])
            issue_bias(m0, 2)
            issue_bias(m0, 3)
            # deferred big loads: (emit_at_blk, fn) issued inside pair 0's loop
            deferred_loads = [
                (2, lambda: nc.gpsimd.dma_start(kT[:, :, 512:768],
                                                kT_ap[:, :, 512:768])),
                (4, lambda: nc.gpsimd.dma_start(kT[:, :, 768:1024],
                                                kT_ap[:, :, 768:1024])),
                (6, lambda: nc.gpsimd.dma_start(v_sb[:, 2:4], v_ap[:, 2:4])),
                (8, lambda: nc.gpsimd.dma_start(v_sb[:, 4:6], v_ap[:, 4:6])),
                (10, lambda: nc.gpsimd.dma_start(v_sb[:, 6:8], v_ap[:, 6:8])),
            ]

            evac_idx = [0]

            def emit_qk_op(mm, dnat_mm, c0, h, pool=None, key=None):
                """One QK matmul + PSUM evacuation for pair mm."""
                Fp = 128 * (2 * mm + 2)
                p0 = (h % 2) * 64
                w = min(512, Fp - c0)
                if pool is None:
                    ps = qk_psum.tile([128, 512], f32, tag="qk")
                elif pool is pm_psum:
                    ps = pool.tile([128, 512], f32, tag="pm")
                else:
                    ps = pool.tile([128, 512], f32, tag="tp")
                nc.tensor.matmul(ps[:, :w],
                                 qT[p0:p0 + 64, mm, h // 2, :],
                                 kT[p0:p0 + 64, h // 2, c0:c0 + w],
                                 start=True, stop=True)
                if key is None:
                    key = qk_evac[evac_idx[0] % len(qk_evac)]
                    evac_idx[0] += 1
                copy_on(key, dnat_mm[:, h, c0:c0 + w], ps[:, :w])

            def qk_ops(mm):
                Fp = 128 * (2 * mm + 2)
                return [(c0, h) for c0 in range(0, Fp, 512) for h in range(H)]

            dnats = {}
            dnats[0] = dnat_pool.tile([128, H, 128 * (2 * m0 + 2)], f16,
                                      tag="dnat0", name="dnat0")
            borrow = {"pm": pm_psum, "tp": tp_psum}.get(FIRST_PM)
            for i, (c0, h) in enumerate(qk_ops(m0)):
                pool = borrow if (borrow is not None and i % 2 == 1 and i < 16) else None
                key = "vs"[i % 2] if i < 18 else None
                emit_qk_op(m0, dnats[0], c0, h, pool=pool, key=key)

            tp_idx = [0]
            shufs = {}

            for mi, m in enumerate(mseq):
                extp = 2 * m + 2          # pair-level j-chunks (max of its blks)
                dnat = dnats.get(mi)
                # software-pipeline: next pair's QK ops interleave with this
                # pair's per-block chain; reserve a few for the AV section.
                nxt = []
                if mi + 1 < len(mseq):
                    mn = mseq[mi + 1]
                    dnats[mi + 1] = dnat_pool.tile(
                        [128, H, 128 * (2 * mn + 2)], f16,
                        tag=f"dnat{(mi + 1) % 2}", name=f"dnat{mi + 1}")
                    nxt = qk_ops(mn)
                nxt_blk = nxt
                per_blk = (len(nxt_blk) + FRONT - 1) // FRONT if nxt_blk else 0

                et = et_pool.tile([128, extp, NBLK * 128], f16,
                                  tag=f"et{mi % 2}", name=f"et{mi}")

                tp_pat = tp_evac[mi] if isinstance(tp_evac, (tuple, list)) \
                    else tp_evac

                def emit_tp(blk, ext, E, R):
                    # --- post-mix + transpose + normalize: out[j,(i8,g)]
                    #     batched: 4 jc per PSUM bank, ONE evac per bank
                    for jq in range(0, ext, 4):
                        nj = min(4, ext - jq)
                        tp = tp_psum.tile([128, 512], f32, tag="tp")
                        for j in range(nj):
                            jc = jq + j
                            nc.tensor.matmul(tp[:, j * 128:(j + 1) * 128],
                                             E[:, jc * 128:(jc + 1) * 128],
                                             R[:], start=True, stop=True)
                        key = tp_pat[tp_idx[0] % len(tp_pat)]
                        tp_idx[0] += 1
                        src = tp[:, :nj * 128].rearrange("p (a b) -> p a b", a=nj)
                        dst = et[:, jq:jq + nj, blk * 128:(blk + 1) * 128]
                        if len(key) == 1:
                            copy_on(key, dst, src)
                        else:
                            # split the evac across engines to free the bank faster
                            hw = (nj + 1) // 2
                            copy_on(key[0], dst[:, :hw], src[:, :hw])
                            copy_on(key[1], dst[:, hw:], src[:, hw:])

                def issue_shuffle(mm, dn, blk):
                    ext = _pair_ext(mm, blk)
                    F = 128 * ext
                    dshuf = dshuf_pool.tile([128, 1024], f16, tag="dshuf")
                    nc.sync.dma_start(dshuf[:, :F],
                                      dn[blk * 8:(blk + 1) * 8, :, :F])
                    shufs[(mm, blk)] = dshuf

                pend = None   # (blk, ext, E, R) deferred by one block
                for blk in range(NBLK + 1):
                    ops = []
                    if blk < NBLK:
                        # prefetch next pair's bias quarters early
                        if blk % 4 == 0 and mi + 1 < len(mseq):
                            issue_bias(mseq[mi + 1], blk // 4)
                        if mi == 0:
                            while deferred_loads and deferred_loads[0][0] <= blk:
                                deferred_loads.pop(0)[1]()
                        ops = list(nxt_blk[blk * per_blk:(blk + 1) * per_blk])
                    # spread next-pair QK ops across the block so each QK
                    # PSUM bank has time to drain before reuse
                    if ops:
                        emit_qk_op(mseq[mi + 1], dnats[mi + 1], *ops[0])
                    if pend is not None:
                        emit_tp(*pend)
                        pend = None
                    if blk == NBLK:
                        break
                    if len(ops) > 1:
                        emit_qk_op(mseq[mi + 1], dnats[mi + 1], *ops[1])
                    ext = _pair_ext(m, blk)
                    F = 128 * ext
                    # --- shuffle [8,(h,j)] -> [(i8,h), j]
                    if (m, blk) not in shufs:
                        issue_shuffle(m, dnat, blk)
                    dshuf = shufs.pop((m, blk))
                    # next pair's first shuffles as soon as its QK is done
                    if EARLY_SHUF and blk >= 13 and mi + 1 < len(mseq):
                        issue_shuffle(mseq[mi + 1], dnats[mi + 1], blk - 13)
                    bias_t = bias_tiles[(m, blk // 4)]
                    b4 = blk % 4
                    # --- bias + pre-mix into PSUM; one exp per pm tile
                    E = e_pool.tile([128, 1024], f16, tag="E")
                    s_parts = []
                    if not exp_split:
                        pm = pm_psum.tile([128, 1024], f32, tag="pm")
                    for c0 in range(0, F, 512):
                        w = min(512, F - c0)
                        if exp_split:
                            pm = pm_psum.tile([128, 512], f32, tag="pm")
                            pmv = pm[:, :w]
                        else:
                            pmv = pm[:, c0:c0 + w]
                        nc.tensor.matmul(pmv, ident[:],
                                         bias_t[:, b4, c0:c0 + w],
                                         start=True, stop=False)
                        nc.tensor.matmul(pmv, wpre[:],
                                         dshuf[:, c0:c0 + w],
                                         start=False, stop=True)
                        if exp_split:
                            sc = s_pool.tile([128, 1], f32, tag=f"Sc{len(s_parts)}")
                            nc.scalar.activation(E[:, c0:c0 + w], pmv, Exp,
                                                 bias=shift[:], accum_out=sc[:])
                            s_parts.append(sc)
                    for c0, h in ops[2:]:
                        emit_qk_op(mseq[mi + 1], dnats[mi + 1], c0, h)
                    if exp_split:
                        if len(s_parts) == 1:
                            S = s_parts[0]
                        else:
                            S = s_pool.tile([128, 1], f32, tag="S")
                            nc.vector.tensor_add(S[:], s_parts[0][:],
                                                 s_parts[1][:])
                    else:
                        S = s_pool.tile([128, 1], f32, tag="Sc0")
                        nc.scalar.activation(E[:, :F], pm[:, :F], Exp,
                                             bias=shift[:], accum_out=S[:])
                    Sr = s_pool.tile([128, 1], f32, tag="Sr")
                    nc.vector.reciprocal(Sr[:], S[:])
                    R = s_pool.tile([128, 128], f16, tag="R")
                    nc.gpsimd.tensor_scalar_mul(R[:], wpost[:], Sr[:])
                    pend = (blk, ext, E, R)

                # --- AV: per (g, jc) accumulate over j chunks; two 8-head
                #     halves share one PSUM bank, freeing a bank for QK.
                etv = et[:].rearrange("p e (blk i8 g) -> p e blk i8 g",
                                      blk=NBLK, i8=8)
                out_t = out_pool.tile([128, H, 64], f16, tag="out")
                for half in range(2):
                    av = tp_psum.tile([128, 8, 64], f32, tag="tp")
                    for gh in range(8):
                        g = half * 8 + gh
                        first = True
                        for jc in range(extp):
                            # blocks whose causal extent covers chunk jc
                            blo = 0 if jc < extp - 1 else 8
                            lhs = etv[:, jc, blo:NBLK, :, g]
                            last = (jc == extp - 1)
                            nc.tensor.matmul(av[blo * 8:, gh, :], lhs,
                                             v_sb[:, jc, g, :],
                                             start=first, stop=last)
                            first = False
                    # rows [0,64) got their last accumulation at jc=extp-2;
                    # start/stop flags only matter for psum has_written (start)
                    copy_on(out_eng[half % len(out_eng)],
                            out_t[:, half * 8:half * 8 + 8, :], av[:])
                    nc.sync.dma_start(out_ap[m, :, half * 8:half * 8 + 8, :],
                                      out_t[:, half * 8:half * 8 + 8, :])

    nc.compile()
    return nc


_NC_CACHE = None


def _get_nc():
    global _NC_CACHE
    if _NC_CACHE is None:
        _NC_CACHE = _build_module()
    return _NC_CACHE


def _host_inputs(q, k, v, attn_bias, w_pre, w_post):
    """Build the 8 per-core input maps."""
    scale = np.float32(D ** -0.5)
    f16 = np.float16
    in_maps = []
    # Kronecker mixing matrices, layout p=(i8,h) -> f=(i8,g)
    wpre128 = np.zeros((128, 128), np.float32)
    wpost128 = np.zeros((128, 128), np.float32)
    for i8 in range(8):
        # premix matmul: out[(i8,g)] = sum_(i8,h) lhsT[(i8,h),(i8,g)] * dots
        wpre128[i8 * 16:(i8 + 1) * 16, i8 * 16:(i8 + 1) * 16] = w_pre.T
        wpost128[i8 * 16:(i8 + 1) * 16, i8 * 16:(i8 + 1) * 16] = w_post.T
    wpre128 = wpre128.astype(np.float16)
    ident = np.eye(128, dtype=f16)

    for c in range(N_CORES):
        b, s = c // 2, c % 2
        rows = _core_rows(s)                      # [512]
        qc = q[b][:, rows, :] * scale             # [H, 512, D]
        qTf = np.transpose(qc, (2, 0, 1)).astype(np.float16)  # [D, H, 512]
        # pack: partition (h%2)*64+d, free (pair, h//2, 128)
        qT = np.empty((128, NPAIR, H // 2, 128), np.float16)
        qTr = qTf.reshape(D, H, NPAIR, 128).transpose(0, 2, 1, 3)  # [D,P,H,128]
        qT[:64] = qTr[:, :, 0::2]
        qT[64:] = qTr[:, :, 1::2]
        kTf = np.transpose(k[b], (2, 0, 1)).astype(np.float16)  # [D,H,N]
        kT = np.empty((128, H // 2, N), np.float16)
        kT[:64] = kTf[:, 0::2]
        kT[64:] = kTf[:, 1::2]
        vv = np.ascontiguousarray(
            np.transpose(v[b].astype(f16), (1, 0, 2)).reshape(8, 128, H, 64)
            .transpose(1, 0, 2, 3))               # [128, 8jc, H, 64]
        m_in = {
            "qT": qT, "kT": kT, "v": np.ascontiguousarray(vv),
            "wpre": wpre128, "wpost": wpost128, "ident": ident,
        }
        # bias per (pair, quarter), shuffled to [(i8,h), (blk4, j)] with mask
        for m in range(NPAIR):
            prow = rows[m * 128:(m + 1) * 128]    # global rows of this pair
            for qq in range(4):
                Fq = _quarter_F(m, qq)
                bt = np.empty((128, 4, Fq), np.float32)
                for b4 in range(4):
                    blk = qq * 4 + b4
                    grows = prow[blk * 8:(blk + 1) * 8]   # 8 global row ids
                    # [8 i8, 16 h, Fq]
                    bb = attn_bias[:, grows, :Fq].transpose(1, 0, 2)
                    jj = np.arange(Fq)[None, None, :]
                    ii = grows[:, None, None]
                    bb = np.where(jj > ii, MASK_VAL, bb)
                    bt[:, b4, :] = bb.reshape(128, Fq)
                m_in[f"bias{m}_{qq}"] = bt.astype(f16)
        in_maps.append(m_in)
    return in_maps


def kernel(q, k, v, attn_bias, w_pre, w_post):
    from concourse.bass_utils import run_bass_kernel_spmd

    q, k, v = np.asarray(q), np.asarray(k), np.asarray(v)
    attn_bias = np.asarray(attn_bias)
    w_pre, w_post = np.asarray(w_pre), np.asarray(w_post)

    nc = _get_nc()
    in_maps = _host_inputs(q, k, v, attn_bias, w_pre, w_post)
    res = run_bass_kernel_spmd(nc, in_maps, list(range(N_CORES)))

    out = np.empty((B, H, N, D), np.float32)
    for c in range(N_CORES):
        b, s = c // 2, c % 2
        rows = _core_rows(s)
        oc = res.results[c]["out"].astype(np.float32)  # [NPAIR, 128, H, 64]
        oc = oc.reshape(NPAIR * 128, H, 64).transpose(1, 0, 2)  # [H, 512, 64]
        out[b][:, rows, :] = oc
    return out


if __name__ == "__main__":
    rng = np.random.default_rng(0)
    qq = rng.standard_normal((B, H, N, D), dtype=np.float32)
    kk = rng.standard_normal((B, H, N, D), dtype=np.float32)
    vv = rng.standard_normal((B, H, N, D), dtype=np.float32)
    bb = rng.standard_normal((H, N, N), dtype=np.float32)
    wp = rng.standard_normal((H, H), dtype=np.float32) / 4
    wq = rng.standard_normal((H, H), dtype=np.float32) / 4
    o = kernel(qq, kk, vv, bb, wp, wq)
    print("ran", o.shape, np.abs(o).mean())


# revision 28
# speedup vs baseline: 1.0375x; 1.0375x over previous
"""Talking-heads causal attention kernel for 8 Trainium2 NeuronCores.

Problem: B=4, H=16, N=1024, D=64 (fp32)
  dots = einsum('bhid,bhjd', q, k) * d**-0.5
  dots = einsum('gh,bhij', w_pre, dots) + attn_bias   (talking heads pre)
  causal mask, fp32 softmax
  attn = einsum('gh,bhij', w_post, attn)              (talking heads post)
  out  = einsum('bhij,bhjd', attn, v)
Sharding: core c = (b, s) with b = c//2, s = c%2. Each core owns query rows
R_s = {128k + 64s + [0,64) : k=0..7} of its batch b (interleaved 64-row
blocks -> identical causal work AND identical program on every core).
The h-mixes are local (all 16 heads on-core); no collectives.

Device pipeline per core (pairs m=0..3 of row-groups, 128 rows each):
  QK^T (f16)    ->  dots in natural [i,(h,j)] layout (PSUM -> dnat SBUF,
                    evac spread over DVE/Act/Pool engines)
  DMA shuffle   ->  [(i8,h), j] interleaved layout (8->128 partition DMA, SP)
  bias via identity-matmul into PSUM + pre-mix Kronecker matmul (I8 (x) w_pre)
  ScalarE exp(x-4) with fused row-sum accum
  post-mix+transpose+normalize as ONE matmul: lhsT=E chunk, rhs=R where
     R = (I8 (x) w_post^T) * (1/S) rowwise  ->  out = attn_mixed^T [j,(i8,g)]
     PSUM evacuated in batched 512-col copies
  AV matmul (fp16) with strided lhsT gather, accumulate over j chunks,
  in two 8-head halves sharing one PSUM bank; av -> out_t f16, DMA on SP.
"""

import numpy as np
import ml_dtypes

B, H, N, D = 4, 16, 1024, 64
N_CORES = 8
NBLK = 16          # 8-row blocks per 128-row pair-group
NPAIR = 4          # pair-groups per core (each 128 rows = 16 blks)

MASK_VAL = np.float32(-60000.0)
EXP_SHIFT = -4.0

# engine-assignment patterns (cycled): v=vector(DVE) s=scalar(Act) g=gpsimd(Pool)
QK_EVAC_PAT = "svvv"
TP_EVAC_PAT = "svvv"
OUT_ENG = "s"
MSEQ = (0, 1, 3, 2)


def _core_rows(s):
    """Global row indices (length 512) owned by core (b, s), pair-major."""
    rows = []
    for m in range(NPAIR):
        for k in (2 * m, 2 * m + 1):
            base = 128 * k + 64 * s
            rows.extend(range(base, base + 64))
    return np.array(rows)  # [512]; pair m -> rows[m*128:(m+1)*128]


def _pair_ext(m, blk):
    """#128-wide j-chunks needed by 8-row block blk of pair m (causal)."""
    k = 2 * m + (blk // 8)          # which 64-row group
    return k + 1


def _quarter_F(m, q):
    """j-extent (cols) of bias quarter q (blocks 4q..4q+3) of pair m."""
    return 128 * (2 * m + q // 2 + 1)


def _build_module(qk_evac=QK_EVAC_PAT, tp_evac=TP_EVAC_PAT, out_eng=OUT_ENG,
                  mseq=MSEQ, qk_bufs=2, pm_bufs=2, exp_split=False,
                  EARLY_SHUF=False, FRONT=13, FIRST_PM=False):
    import concourse.bass as bass
    import concourse.mybir as mybir
    import concourse.tile as tile
    from concourse import bacc

    f32, f16 = mybir.dt.float32, mybir.dt.float16

    nc = bacc.Bacc("TRN2", target_bir_lowering=False, debug=False,
                   num_devices=N_CORES)

    # q/k transposed, two heads packed per partition-column: head h lives at
    # partitions (h%2)*64 + d, free index h//2.  qT pair-major for split loads.
    qT_ap = nc.dram_tensor("qT", [128, NPAIR, H // 2, 128], f16, kind="ExternalInput").ap()
    kT_ap = nc.dram_tensor("kT", [128, H // 2, N], f16, kind="ExternalInput").ap()
    v_ap = nc.dram_tensor("v", [128, 8, H, 64], f16, kind="ExternalInput").ap()
    # bias per (pair, quarter): blocks 4q..4q+3 shuffled to [(i8,h), (blk4, j)]
    bias_aps = {}
    for m in range(NPAIR):
        for q in range(4):
            Fq = _quarter_F(m, q)
            bias_aps[(m, q)] = nc.dram_tensor(
                f"bias{m}_{q}", [128, 4, Fq], f16, kind="ExternalInput").ap()
    wpre_ap = nc.dram_tensor("wpre", [128, 128], f16, kind="ExternalInput").ap()
    wpost_ap = nc.dram_tensor("wpost", [128, 128], f32, kind="ExternalInput").ap()
    ident_ap = nc.dram_tensor("ident", [128, 128], f16, kind="ExternalInput").ap()
    out_ap = nc.dram_tensor("out", [NPAIR, 128, H, 64], f16, kind="ExternalOutput").ap()

    with tile.TileContext(nc) as tc:
        with (
            tc.tile_pool(name="const", bufs=1) as cpool,
            tc.tile_pool(name="dnat", bufs=1) as dnat_pool,
            tc.tile_pool(name="dshuf", bufs=4) as dshuf_pool,
            tc.tile_pool(name="ebuf", bufs=4) as e_pool,
            tc.tile_pool(name="et", bufs=1) as et_pool,
            tc.tile_pool(name="biasb", bufs=4) as bias_pool,
            tc.tile_pool(name="small", bufs=4) as s_pool,
            tc.tile_pool(name="outb", bufs=2) as out_pool,
            tc.tile_pool(name="qkps", bufs=qk_bufs, space="PSUM") as qk_psum,
            tc.tile_pool(name="pmps", bufs=pm_bufs, space="PSUM") as pm_psum,
            tc.tile_pool(name="tpps", bufs=2, space="PSUM") as tp_psum,
            # pm tiles are [128,1024] (2 banks) unless exp_split
        ):
            Exp = mybir.ActivationFunctionType.Exp
            ENG = {}

            def copy_on(key, dst, src):
                eng = ENG[key]
                if eng is nc.scalar:
                    eng.copy(dst, src)
                else:
                    eng.tensor_copy(dst, src)

            ENG.update(v=nc.vector, g=nc.gpsimd)
            ENG['s'] = nc.scalar

            # --- constants / inputs, ordered for fast pipeline start: tiny
            # consts on SP (fast HWDGE gen), kT chunk then qT(first pair) on
            # Pool so the first QK + premix unblock ASAP; v late and split.
            m0 = mseq[0]
            wpre = cpool.tile([128, 128], f16, tag="wpre")
            nc.sync.dma_start(wpre[:], wpre_ap[:])
            ident = cpool.tile([128, 128], f16, tag="ident")
            nc.sync.dma_start(ident[:], ident_ap[:])
            kT = cpool.tile([128, H // 2, N], f16, tag="kT")
            F0 = 128 * (2 * m0 + 2)
            nc.gpsimd.dma_start(kT[:, :, 0:F0], kT_ap[:, :, 0:F0])
            qT = cpool.tile([128, NPAIR, H // 2, 128], f16, tag="qT")
            nc.gpsimd.dma_start(qT[:, m0], qT_ap[:, m0])
            wpost = cpool.tile([128, 128], f32, tag="wpost")
            nc.sync.dma_start(wpost[:], wpost_ap[:])
            shift = cpool.tile([128, 1], f32, tag="shift")
            nc.vector.memset(shift[:], EXP_SHIFT)

            bias_tiles = {}

            def issue_bias(m, q):
                Fq = _quarter_F(m, q)
                bt = bias_pool.tile([128, 4, 1024], f16, tag="bias",
                                    name=f"bias{m}_{q}")
                nc.sync.dma_start(bt[:, :, :Fq], bias_aps[(m, q)][:])
                bias_tiles[(m, q)] = bt

            issue_bias(m0, 0)
            for mm in mseq[1:]:
                nc.gpsimd.dma_start(qT[:, mm], qT_ap[:, mm])
            issue_bias(m0, 1)
            kT1 = min(512, N)
            if F0 < kT1:
                nc.gpsimd.dma_start(kT[:, :, F0:kT1], kT_ap[:, :, F0:kT1])
            v_sb = cpool.tile([128, 8, H, 64], f16, tag="v")
            nc.sync.dma_start(v_sb[:, 0:2], v_ap[:, 0:2])
            issue_bias(m0, 2)
            issue_bias(m0, 3)
            # deferred big loads: (emit_at_blk, fn) issued inside pair 0's loop
            deferred_loads = [
                (2, lambda: nc.gpsimd.dma_start(kT[:, :, 512:768],
                                                kT_ap[:, :, 512:768])),
                (4, lambda: nc.gpsimd.dma_start(kT[:, :, 768:1024],
                                                kT_ap[:, :, 768:1024])),
                (6, lambda: nc.gpsimd.dma_start(v_sb[:, 2:4], v_ap[:, 2:4])),
                (8, lambda: nc.gpsimd.dma_start(v_sb[:, 4:6], v_ap[:, 4:6])),
                (10, lambda: nc.gpsimd.dma_start(v_sb[:, 6:8], v_ap[:, 6:8])),
            ]

            evac_idx = [0]

            def emit_qk_op(mm, dnat_mm, c0, h, pool=None, key=None):
                """One QK matmul + PSUM evacuation for pair mm."""
                Fp = 128 * (2 * mm + 2)
                p0 = (h % 2) * 64
                w = min(512, Fp - c0)
                if pool is None:
                    ps = qk_psum.tile([128, 512], f32, tag="qk")
                elif pool is pm_psum:
                    ps = pool.tile([128, 512], f32, tag="pm")
                else:
                    ps = pool.tile([128, 512], f32, tag="tp")
                nc.tensor.matmul(ps[:, :w],
                                 qT[p0:p0 + 64, mm, h // 2, :],
                                 kT[p0:p0 + 64, h // 2, c0:c0 + w],
                                 start=True, stop=True)
                if key is None:
                    key = qk_evac[evac_idx[0] % len(qk_evac)]
                    evac_idx[0] += 1
                copy_on(key, dnat_mm[:, h, c0:c0 + w], ps[:, :w])

            def qk_ops(mm):
                Fp = 128 * (2 * mm + 2)
                return [(c0, h) for c0 in range(0, Fp, 512) for h in range(H)]

            dnats = {}
            dnats[0] = dnat_pool.tile([128, H, 128 * (2 * m0 + 2)], f16,
                                      tag="dnat0", name="dnat0")
            borrow = {"pm": pm_psum, "tp": tp_psum}.get(FIRST_PM)
            for i, (c0, h) in enumerate(qk_ops(m0)):
                pool = borrow if (borrow is not None and i % 2 == 1 and i < 16) else None
                key = "vs"[i % 2] if i < 18 else None
                emit_qk_op(m0, dnats[0], c0, h, pool=pool, key=key)

            tp_idx = [0]
            shufs = {}

            for mi, m in enumerate(mseq):
                extp = 2 * m + 2          # pair-level j-chunks (max of its blks)
                dnat = dnats.get(mi)
                # software-pipeline: next pair's QK ops interleave with this
                # pair's per-block chain; reserve a few for the AV section.
                nxt = []
                if mi + 1 < len(mseq):
                    mn = mseq[mi + 1]
                    dnats[mi + 1] = dnat_pool.tile(
                        [128, H, 128 * (2 * mn + 2)], f16,
                        tag=f"dnat{(mi + 1) % 2}", name=f"dnat{mi + 1}")
                    nxt = qk_ops(mn)
                nxt_blk = nxt
                per_blk = (len(nxt_blk) + FRONT - 1) // FRONT if nxt_blk else 0

                et = et_pool.tile([128, extp, NBLK * 128], f16,
                                  tag=f"et{mi % 2}", name=f"et{mi}")

                tp_pat = tp_evac[mi] if isinstance(tp_evac, (tuple, list)) \
                    else tp_evac

                def emit_tp(blk, ext, E, R):
                    # --- post-mix + transpose + normalize: out[j,(i8,g)]
                    #     batched: 4 jc per PSUM bank, ONE evac per bank
                    for jq in range(0, ext, 4):
                        nj = min(4, ext - jq)
                        tp = tp_psum.tile([128, 512], f32, tag="tp")
                        for j in range(nj):
                            jc = jq + j
                            nc.tensor.matmul(tp[:, j * 128:(j + 1) * 128],
                                             E[:, jc * 128:(jc + 1) * 128],
                                             R[:], start=True, stop=True)
                        key = tp_pat[tp_idx[0] % len(tp_pat)]
                        tp_idx[0] += 1
                        src = tp[:, :nj * 128].rearrange("p (a b) -> p a b", a=nj)
                        dst = et[:, jq:jq + nj, blk * 128:(blk + 1) * 128]
                        if len(key) == 1:
                            copy_on(key, dst, src)
                        else:
                            # split the evac across engines to free the bank faster
                            hw = (nj + 1) // 2
                            copy_on(key[0], dst[:, :hw], src[:, :hw])
                            copy_on(key[1], dst[:, hw:], src[:, hw:])

                def issue_shuffle(mm, dn, blk):
                    ext = _pair_ext(mm, blk)
                    F = 128 * ext
                    dshuf = dshuf_pool.tile([128, 1024], f16, tag="dshuf")
                    nc.sync.dma_start(dshuf[:, :F],
                                      dn[blk * 8:(blk + 1) * 8, :, :F])
                    shufs[(mm, blk)] = dshuf

                pend = None   # (blk, ext, E, R) deferred by one block
                for blk in range(NBLK + 1):
                    ops = []
                    if blk < NBLK:
                        # prefetch next pair's bias quarters early
                        if blk % 4 == 0 and mi + 1 < len(mseq):
                            issue_bias(mseq[mi + 1], blk // 4)
                        if mi == 0:
                            while deferred_loads and deferred_loads[0][0] <= blk:
                                deferred_loads.pop(0)[1]()
                        ops = list(nxt_blk[blk * per_blk:(blk + 1) * per_blk])
                    # spread next-pair QK ops across the block so each QK
                    # PSUM bank has time to drain before reuse
                    if ops:
                        emit_qk_op(mseq[mi + 1], dnats[mi + 1], *ops[0])
                    if pend is not None:
                        emit_tp(*pend)
                        pend = None
                    if blk == NBLK:
                        break
                    if len(ops) > 1:
                        emit_qk_op(mseq[mi + 1], dnats[mi + 1], *ops[1])
                    ext = _pair_ext(m, blk)
                    F = 128 * ext
                    # --- shuffle [8,(h,j)] -> [(i8,h), j]
                    if (m, blk) not in shufs:
                        issue_shuffle(m, dnat, blk)
                    dshuf = shufs.pop((m, blk))
                    # next pair's first shuffles as soon as its QK is done
                    if EARLY_SHUF and blk >= 13 and mi + 1 < len(mseq):
                        issue_shuffle(mseq[mi + 1], dnats[mi + 1], blk - 13)
                    bias_t = bias_tiles[(m, blk // 4)]
                    b4 = blk % 4
                    # --- bias + pre-mix into PSUM; one exp per pm tile
                    E = e_pool.tile([128, 1024], f16, tag="E")
                    s_parts = []
                    if not exp_split:
                        pm = pm_psum.tile([128, 1024], f32, tag="pm")
                    for c0 in range(0, F, 512):
                        w = min(512, F - c0)
                        if exp_split:
                            pm = pm_psum.tile([128, 512], f32, tag="pm")
                            pmv = pm[:, :w]
                        else:
                            pmv = pm[:, c0:c0 + w]
                        nc.tensor.matmul(pmv, ident[:],
                                         bias_t[:, b4, c0:c0 + w],
                                         start=True, stop=False)
                        nc.tensor.matmul(pmv, wpre[:],
                                         dshuf[:, c0:c0 + w],
                                         start=False, stop=True)
                        if exp_split:
                            sc = s_pool.tile([128, 1], f32, tag=f"Sc{len(s_parts)}")
                            nc.scalar.activation(E[:, c0:c0 + w], pmv, Exp,
                                                 bias=shift[:], accum_out=sc[:])
                            s_parts.append(sc)
                    for c0, h in ops[2:]:
                        emit_qk_op(mseq[mi + 1], dnats[mi + 1], c0, h)
                    if exp_split:
                        if len(s_parts) == 1:
                            S = s_parts[0]
                        else:
                            S = s_pool.tile([128, 1], f32, tag="S")
                            nc.vector.tensor_add(S[:], s_parts[0][:],
                                                 s_parts[1][:])
                    else:
                        S = s_pool.tile([128, 1], f32, tag="Sc0")
                        nc.scalar.activation(E[:, :F], pm[:, :F], Exp,
                                             bias=shift[:], accum_out=S[:])
                    Sr = s_pool.tile([128, 1], f32, tag="Sr")
                    nc.vector.reciprocal(Sr[:], S[:])
                    R = s_pool.tile([128, 128], f16, tag="R")
                    nc.gpsimd.tensor_scalar_mul(R[:], wpost[:], Sr[:])
                    pend = (blk, ext, E, R)

                # --- AV: per (g, jc) accumulate over j chunks; two 8-head
                #     halves share one PSUM bank, freeing a bank for QK.
                etv = et[:].rearrange("p e (blk i8 g) -> p e blk i8 g",
                                      blk=NBLK, i8=8)
                out_t = out_pool.tile([128, H, 64], f16, tag="out")
                for half in range(2):
                    av = tp_psum.tile([128, 8, 64], f32, tag="tp")
                    for gh in range(8):
                        g = half * 8 + gh
                        first = True
                        for jc in range(extp):
                            # blocks whose causal extent covers chunk jc
                            blo = 0 if jc < extp - 1 else 8
                            lhs = etv[:, jc, blo:NBLK, :, g]
                            last = (jc == extp - 1)
                            nc.tensor.matmul(av[blo * 8:, gh, :], lhs,
                                             v_sb[:, jc, g, :],
                                             start=first, stop=last)
                            first = False
                    # rows [0,64) got their last accumulation at jc=extp-2;
                    # start/stop flags only matter for psum has_written (start)
                    copy_on(out_eng[half % len(out_eng)],
                            out_t[:, half * 8:half * 8 + 8, :], av[:])
                    nc.sync.dma_start(out_ap[m, :, half * 8:half * 8 + 8, :],
                                      out_t[:, half * 8:half * 8 + 8, :])

    nc.compile()
    return nc


_NC_CACHE = None


def _get_nc():
    global _NC_CACHE
    if _NC_CACHE is None:
        _NC_CACHE = _build_module()
    return _NC_CACHE


def _host_inputs(q, k, v, attn_bias, w_pre, w_post):
    """Build the 8 per-core input maps."""
    scale = np.float32(D ** -0.5)
    f16 = np.float16
    in_maps = []
    # Kronecker mixing matrices, layout p=(i8,h) -> f=(i8,g)
    wpre128 = np.zeros((128, 128), np.float32)
    wpost128 = np.zeros((128, 128), np.float32)
    for i8 in range(8):
        # premix matmul: out[(i8,g)] = sum_(i8,h) lhsT[(i8,h),(i8,g)] * dots
        wpre128[i8 * 16:(i8 + 1) * 16, i8 * 16:(i8 + 1) * 16] = w_pre.T
        wpost128[i8 * 16:(i8 + 1) * 16, i8 * 16:(i8 + 1) * 16] = w_post.T
    wpre128 = wpre128.astype(np.float16)
    ident = np.eye(128, dtype=f16)

    for c in range(N_CORES):
        b, s = c // 2, c % 2
        rows = _core_rows(s)                      # [512]
        qc = q[b][:, rows, :] * scale             # [H, 512, D]
        qTf = np.transpose(qc, (2, 0, 1)).astype(np.float16)  # [D, H, 512]
        # pack: partition (h%2)*64+d, free (pair, h//2, 128)
        qT = np.empty((128, NPAIR, H // 2, 128), np.float16)
        qTr = qTf.reshape(D, H, NPAIR, 128).transpose(0, 2, 1, 3)  # [D,P,H,128]
        qT[:64] = qTr[:, :, 0::2]
        qT[64:] = qTr[:, :, 1::2]
        kTf = np.transpose(k[b], (2, 0, 1)).astype(np.float16)  # [D,H,N]
        kT = np.empty((128, H // 2, N), np.float16)
        kT[:64] = kTf[:, 0::2]
        kT[64:] = kTf[:, 1::2]
        vv = np.ascontiguousarray(
            np.transpose(v[b].astype(f16), (1, 0, 2)).reshape(8, 128, H, 64)
            .transpose(1, 0, 2, 3))               # [128, 8jc, H, 64]
        m_in = {
            "qT": qT, "kT": kT, "v": np.ascontiguousarray(vv),
            "wpre": wpre128, "wpost": wpost128, "ident": ident,
        }
        # bias per (pair, quarter), shuffled to [(i8,h), (blk4, j)] with mask
        for m in range(NPAIR):
            prow = rows[m * 128:(m + 1) * 128]    # global rows of this pair
            for qq in range(4):
                Fq = _quarter_F(m, qq)
                bt = np.empty((128, 4, Fq), np.float32)
                for b4 in range(4):
                    blk = qq * 4 + b4
                    grows = prow[blk * 8:(blk + 1) * 8]   # 8 global row ids
                    # [8 i8, 16 h, Fq]
                    bb = attn_bias[:, grows, :Fq].transpose(1, 0, 2)
                    jj = np.arange(Fq)[None, None, :]
                    ii = grows[:, None, None]
                    bb = np.where(jj > ii, MASK_VAL, bb)
                    bt[:, b4, :] = bb.reshape(128, Fq)
                m_in[f"bias{m}_{qq}"] = bt.astype(f16)
        in_maps.append(m_in)
    return in_maps


def kernel(q, k, v, attn_bias, w_pre, w_post):
    from concourse.bass_utils import run_bass_kernel_spmd

    q, k, v = np.asarray(q), np.asarray(k), np.asarray(v)
    attn_bias = np.asarray(attn_bias)
    w_pre, w_post = np.asarray(w_pre), np.asarray(w_post)

    nc = _get_nc()
    in_maps = _host_inputs(q, k, v, attn_bias, w_pre, w_post)
    res = run_bass_kernel_spmd(nc, in_maps, list(range(N_CORES)))

    out = np.empty((B, H, N, D), np.float32)
    for c in range(N_CORES):
        b, s = c // 2, c % 2
        rows = _core_rows(s)
        oc = res.results[c]["out"].astype(np.float32)  # [NPAIR, 128, H, 64]
        oc = oc.reshape(NPAIR * 128, H, 64).transpose(1, 0, 2)  # [H, 512, 64]
        out[b][:, rows, :] = oc
    return out


if __name__ == "__main__":
    rng = np.random.default_rng(0)
    qq = rng.standard_normal((B, H, N, D), dtype=np.float32)
    kk = rng.standard_normal((B, H, N, D), dtype=np.float32)
    vv = rng.standard_normal((B, H, N, D), dtype=np.float32)
    bb = rng.standard_normal((H, N, N), dtype=np.float32)
    wp = rng.standard_normal((H, H), dtype=np.float32) / 4
    wq = rng.standard_normal((H, H), dtype=np.float32) / 4
    o = kernel(qq, kk, vv, bb, wp, wq)
    print("ran", o.shape, np.abs(o).mean())
